# revision 1
# baseline (speedup 1.0000x reference)
"""GNN bi-interaction aggregator kernel for 8 TRN2 NeuronCores.

Reference computation:
    side = entity_embed[src] * att            # [E, D] gather + edge scale
    N_h  = segment_sum(side, dst, N)          # [N, D] scatter-add
    out  = lrelu((x + N_h) @ W1.T + b1) + lrelu((x * N_h) @ W2.T + b2)

Device strategy (per core, nodes partitioned across 8 cores):
  - Sources are 2-colored (A/B) so each side fits a <=32768-row gather table
    (dma_gather indices are int16).  The coloring balances each destination
    node's A/B in-degree so per-tile chunk counts stay tight.
  - Destination nodes are sorted by (degA, degB) desc and packed into
    128-lane tiles; tile t gets K_A[t] A-chunks and K_B[t] B-chunks.  Edge
    j of node n (within one color) sits at lane n%128 of chunk j, so the
    scatter matrix is the constant identity: PSUM accumulation of
    transpose-mode matmuls computes the segment sum (feature-major).
  - att is applied with one batched DVE tensor_tensor multiply per group of
    4 tiles (broadcast access pattern along the feature axis).
  - Final phase per 4-tile group (512 node columns): sum/prod with x^T,
    two 128x128 matmuls (weights pre-transposed on host), Lrelu with
    per-partition bias on ACT, final add on DVE, DMA out.

Output is produced feature-major per core ([128, NT*128]) and re-assembled
(transpose + inverse node permutation) on the host.
"""

import math
import os
from contextlib import ExitStack
from dataclasses import dataclass, field

import numpy as np

import concourse.bacc as bacc
import concourse.bass as bass
import concourse.mybir as mybir
from concourse.bass_utils import run_bass_kernel_spmd
from concourse.library_config import mlp

F32 = mybir.dt.float32
I16 = mybir.dt.int16
LANES = 128
D = 128
GROUP = 4  # tiles per supergroup (phase-2 batch of 512 columns)
IDX16_MAX = 32768  # rows per gather table (int16 index limit)


# --------------------------------------------------------------------------
# Host-side planning
# --------------------------------------------------------------------------

@dataclass
class Plan:
    n_nodes: int
    n_edges: int
    n_cores: int
    NT: int                 # tiles per core
    NG: int                 # groups per core (NT / GROUP)
    NPAD: int               # padded node count = cores * NT * 128
    nA: int                 # rows in table A
    K_A: np.ndarray         # [NT] A-chunks per tile (device order)
    K_B: np.ndarray         # [NT] B-chunks per tile
    nAg: np.ndarray         # [NG] A-chunks per group
    nBg: np.ndarray         # [NG] B-chunks per group
    a_base: np.ndarray      # [NT] chunk base of tile's A-chunks within group
    b_base: np.ndarray      # [NT] chunk base of tile's B-chunks within group B part
    att_off: np.ndarray     # [NG] att column offset per group
    idxA_off: np.ndarray    # [NG] A-chunk offset per group (global A order)
    idxB_off: np.ndarray    # [NG] B-chunk offset
    NCH_TOT: int
    NCH_MAX: int
    totA: int               # total A gather indices per core
    totB: int
    node_ids: np.ndarray    # [cores, NT*128] node id at (core, tile, lane)
    table: np.ndarray       # [NPAD, 128] f32 permuted embedding table
    inputs: list            # per-core dict of input arrays
    # passthrough for assembly
    order: np.ndarray = field(default=None)


def _color_sources(src, dst, n_real, npad, rng):
    """2-color sources balancing each dst's A/B in-degree.

    Returns boolean colorA[npad]."""
    eorder = np.argsort(src, kind="stable")
    ssorted = src[eorder]
    dsorted = dst[eorder]
    # boundaries per source
    uniq, starts = np.unique(ssorted, return_index=True)
    ends = np.append(starts[1:], len(ssorted))
    outdeg = ends - starts
    # process sources in decreasing out-degree
    proc = np.argsort(-outdeg, kind="stable")
    imb = np.zeros(npad, np.int64)
    colorA = np.zeros(npad, bool)
    capA = IDX16_MAX
    capB = IDX16_MAX
    nA = 0
    nB = 0
    n_sources = len(uniq)
    for qi in proc:
        s = uniq[qi]
        d = dsorted[starts[qi]:ends[qi]]
        sA = imb[d].sum()
        chooseA = sA <= 0
        if chooseA and nA >= capA - (npad - n_real):
            chooseA = False
        if (not chooseA) and nB >= capB - (npad - n_real):
            chooseA = True
        colorA[s] = chooseA
        if chooseA:
            np.add.at(imb, d, 1)
            nA += 1
        else:
            np.add.at(imb, d, -1)
            nB += 1
    # color the remaining (edge-less + padding) nodes to balance counts
    rest = np.ones(npad, bool)
    rest[uniq] = False
    rest_ids = np.nonzero(rest)[0]
    target_A = min(capA, max(npad - capB, npad // 2))
    needA = max(0, target_A - nA)
    needA = min(needA, len(rest_ids))
    colorA[rest_ids[:needA]] = True
    return colorA


def make_plan(entity_embed, att, src, dst, n_cores=8, seed=0):
    n_real, d = entity_embed.shape
    assert d == D
    E = len(src)
    rng = np.random.default_rng(seed)

    nodes_per_core_unit = n_cores * LANES * GROUP
    NT = GROUP * math.ceil((n_real + 2 * n_cores) / nodes_per_core_unit)
    NG = NT // GROUP
    NPAD = n_cores * NT * LANES
    assert NPAD - n_real >= 2, "need at least 2 padding nodes for zero rows"
    assert NPAD <= 2 * IDX16_MAX, "node count exceeds two int16 tables"

    src = np.asarray(src, np.int64)
    dst = np.asarray(dst, np.int64)
    attf = np.asarray(att, np.float32).reshape(-1)

    colorA = _color_sources(src, dst, n_real, NPAD, rng)
    nA = int(colorA.sum())
    nB = NPAD - nA
    assert nA <= IDX16_MAX and nB <= IDX16_MAX, (nA, nB)
    # ensure at least one padding (zero) node on each side
    pad_ids = np.arange(n_real, NPAD)
    assert colorA[pad_ids].any() and (~colorA[pad_ids]).any(), \
        "padding nodes must land on both table sides"

    # table rows: A nodes first, then B nodes
    tabrow = np.empty(NPAD, np.int64)
    a_ids = np.nonzero(colorA)[0]
    b_ids = np.nonzero(~colorA)[0]
    tabrow[a_ids] = np.arange(nA)
    tabrow[b_ids] = nA + np.arange(nB)
    table = np.zeros((NPAD, D), np.float32)
    table[tabrow[:n_real]] = np.asarray(entity_embed, np.float32)

    zeroA = int(tabrow[pad_ids[colorA[pad_ids]][0]])           # A-table row
    zeroB = int(tabrow[pad_ids[~colorA[pad_ids]][0]] - nA)     # B-table row

    # per-destination colored degrees
    isA_e = colorA[src]
    degA = np.bincount(dst[isA_e], minlength=NPAD)
    degB = np.bincount(dst[~isA_e], minlength=NPAD)

    # node ordering: (degA desc, degB desc)
    order = np.lexsort((-degB, -degA))
    inv_pos = np.empty(NPAD, np.int64)
    inv_pos[order] = np.arange(NPAD)

    # maps: sorted position -> (core, t_order, tau, lane)
    # global tile g = pos // 128 ; core = g % C ; t_order = g // C
    # tau = (t_order % NG) * GROUP + t_order // NG
    C = n_cores

    def tau_of_torder(t_order):
        return (t_order % NG) * GROUP + t_order // NG

    def torder_of_tau(tau):
        return (tau % GROUP) * NG + tau // GROUP

    # per-(core,tau) node ids
    node_ids = np.empty((C, NT * LANES), np.int64)
    for c in range(C):
        for tau in range(NT):
            g = torder_of_tau(tau) * C + c
            node_ids[c, tau * LANES:(tau + 1) * LANES] = \
                order[g * LANES:(g + 1) * LANES]

    # K profiles (shared across cores): max colored degree over the 8 tiles
    K_A = np.zeros(NT, np.int64)
    K_B = np.zeros(NT, np.int64)
    for tau in range(NT):
        ids = node_ids[:, tau * LANES:(tau + 1) * LANES].reshape(-1)
        K_A[tau] = max(1, int(degA[ids].max()))
        K_B[tau] = int(degB[ids].max())

    # group chunk layout
    nAg = np.array([K_A[g * GROUP:(g + 1) * GROUP].sum() for g in range(NG)])
    nBg = np.array([K_B[g * GROUP:(g + 1) * GROUP].sum() for g in range(NG)])
    a_base = np.zeros(NT, np.int64)
    b_base = np.zeros(NT, np.int64)
    for g in range(NG):
        accA = accB = 0
        for tau in range(g * GROUP, (g + 1) * GROUP):
            a_base[tau] = accA
            b_base[tau] = accB
            accA += K_A[tau]
            accB += K_B[tau]
    nch_g = nAg + nBg
    att_off = np.concatenate([[0], np.cumsum(nch_g)[:-1]])
    idxA_off = np.concatenate([[0], np.cumsum(nAg)[:-1]])
    idxB_off = np.concatenate([[0], np.cumsum(nBg)[:-1]])
    NCH_TOT = int(nch_g.sum())
    NCH_MAX = int(nch_g.max())
    totA = int(nAg.sum()) * LANES
    totB = int(nBg.sum()) * LANES

    # --- per-edge slot assignment -----------------------------------------
    pos = inv_pos[dst]
    lane = pos % LANES
    gtile = pos // LANES
    core = gtile % C
    t_order = gtile // C
    tau = (t_order % NG) * GROUP + (t_order // NG)
    grp = tau // GROUP

    # rank of edge within (dst, color): stable sort by (pos, colorB)
    key = pos * 2 + (~isA_e).astype(np.int64)
    korder = np.argsort(key, kind="stable")
    ks = key[korder]
    newgrp = np.ones(E, bool)
    newgrp[1:] = ks[1:] != ks[:-1]
    grp_start = np.nonzero(newgrp)[0]
    grp_id = np.cumsum(newgrp) - 1
    rank_sorted = np.arange(E) - grp_start[grp_id]
    rank = np.empty(E, np.int64)
    rank[korder] = rank_sorted

    # gather index flat position + att column
    chunkA = idxA_off[grp] + a_base[tau] + rank            # valid where isA_e
    chunkB = idxB_off[grp] + b_base[tau] + rank
    attcolA = att_off[grp] + a_base[tau] + rank
    attcolB = att_off[grp] + nAg[grp] + b_base[tau] + rank

    srcrow = tabrow[src]          # row in table (A rows < nA)
    srcrowB = srcrow - nA

    inputs = []
    for c in range(C):
        mA = (core == c) & isA_e
        mB = (core == c) & (~isA_e)
        idxA_flat = np.full(totA, zeroA, np.int16)
        idxB_flat = np.full(totB, zeroB, np.int16)
        att_arr = np.zeros((LANES, max(NCH_TOT, 1)), np.float32)
        idxA_flat[(chunkA[mA] * LANES + lane[mA])] = srcrow[mA].astype(np.int16)
        idxB_flat[(chunkB[mB] * LANES + lane[mB])] = srcrowB[mB].astype(np.int16)
        att_arr[lane[mA], attcolA[mA]] = attf[mA]
        att_arr[lane[mB], attcolB[mB]] = attf[mB]

        # wrap idx arrays: element i -> [i%16 + 16*r, i//16] for r in 0..7
        def wrap(flat):
            if len(flat) == 0:
                return np.zeros((128, 1), np.int16)
            w = flat.reshape(-1, 16).T  # [16, n/16]
            return np.tile(w, (8, 1)).copy()

        xT = table[tabrow[node_ids[c]]].T.copy()  # [128, NT*128]
        inputs.append({
            "table": table,
            "xT": np.ascontiguousarray(xT, np.float32),
            "idxA": wrap(idxA_flat),
            "idxB": wrap(idxB_flat),
            "attw": att_arr,
        })

    return Plan(
        n_nodes=n_real, n_edges=E, n_cores=C, NT=NT, NG=NG, NPAD=NPAD, nA=nA,
        K_A=K_A, K_B=K_B, nAg=nAg, nBg=nBg, a_base=a_base, b_base=b_base,
        att_off=att_off, idxA_off=idxA_off, idxB_off=idxB_off,
        NCH_TOT=NCH_TOT, NCH_MAX=NCH_MAX, totA=totA, totB=totB,
        node_ids=node_ids, table=table, inputs=inputs, order=order,
    )


# --------------------------------------------------------------------------
# Bass program
# --------------------------------------------------------------------------

def build_nc(plan, W1, b1, W2, b2, debug=False, ident_bf16=False):
    STAGE = int(os.environ.get("KSTAGE", "4"))
    """Build the SPMD bass program. Weight/bias constants are baked as
    inputs shared by all cores (appended to each in_map by caller)."""
    p = plan
    NT, NG = p.NT, p.NG
    nB_rows = p.NPAD - p.nA

    nc = bacc.Bacc("TRN2", target_bir_lowering=False, debug=debug)
    AF = mybir.ActivationFunctionType

    table = nc.dram_tensor("table", [p.NPAD, D], F32, kind="ExternalInput")
    xT_d = nc.dram_tensor("xT", [LANES, NT * LANES], F32, kind="ExternalInput")
    idxA_d = nc.dram_tensor("idxA", [LANES, max(p.totA // 16, 1)], I16,
                            kind="ExternalInput")
    idxB_d = nc.dram_tensor("idxB", [LANES, max(p.totB // 16, 1)], I16,
                            kind="ExternalInput")
    att_d = nc.dram_tensor("attw", [LANES, max(p.NCH_TOT, 1)], F32,
                           kind="ExternalInput")
    w1_d = nc.dram_tensor("W1t", [D, D], F32, kind="ExternalInput")
    w2_d = nc.dram_tensor("W2t", [D, D], F32, kind="ExternalInput")
    b1_d = nc.dram_tensor("b1c", [D, 1], F32, kind="ExternalInput")
    b2_d = nc.dram_tensor("b2c", [D, 1], F32, kind="ExternalInput")
    b1n_d = nc.dram_tensor("b1n", [D, 1], F32, kind="ExternalInput")
    b2n_d = nc.dram_tensor("b2n", [D, 1], F32, kind="ExternalInput")
    id_dt = mybir.dt.bfloat16 if ident_bf16 else F32
    ident_d = nc.dram_tensor("ident", [D, D], id_dt, kind="ExternalInput")
    outT_d = nc.dram_tensor("outT", [LANES, NT * LANES], F32,
                            kind="ExternalOutput")

    GW = 512  # phase-2 group width (GROUP * 128)

    with ExitStack() as ctx:
        sb = lambda name, shape, dt=F32: ctx.enter_context(
            nc.sbuf_tensor(name, shape, dt))
        ps = lambda name, shape: ctx.enter_context(
            nc.psum_tensor(name, shape, F32))
        sem = lambda name: ctx.enter_context(nc.semaphore(name))

        xT = sb("xT_sb", [LANES, NT * LANES])
        idxA = sb("idxA_sb", [LANES, max(p.totA // 16, 1)], I16)
        idxB = sb("idxB_sb", [LANES, max(p.totB // 16, 1)], I16)
        attw = sb("att_sb", [LANES, max(p.NCH_TOT, 1)])
        w1s = sb("w1_sb", [D, D])
        w2s = sb("w2_sb", [D, D])
        b1s = sb("b1_sb", [D, 1])
        b2s = sb("b2_sb", [D, 1])
        b1ns = sb("b1n_sb", [D, 1])
        b2ns = sb("b2n_sb", [D, 1])
        idents = sb("ident_sb", [D, D], id_dt)
        gbuf = [sb(f"gbuf{i}", [LANES, p.NCH_MAX, D]) for i in range(2)]
        nhb = [sb(f"nh{i}", [LANES, GW]) for i in range(2)]
        sumb = [sb(f"sum{i}", [LANES, GW]) for i in range(2)]
        prodb = [sb(f"prod{i}", [LANES, GW]) for i in range(2)]
        t1b = [sb(f"t1_{i}", [LANES, GW]) for i in range(2)]
        t2b = [sb(f"t2_{i}", [LANES, GW]) for i in range(2)]
        u1b = [sb(f"u1_{i}", [LANES, GW]) for i in range(2)]
        u2b = [sb(f"u2_{i}", [LANES, GW]) for i in range(2)]
        wb = sb("wb_scratch", [LANES, GW])
        outb = [sb(f"outb{i}", [LANES, GW]) for i in range(2)]

        # PSUM: 4 accumulator banks + 2 z banks (each its own bank)
        acc = [ps(f"acc{i}", [LANES, GW]) for i in range(4)]
        zb1 = ps("zb1", [LANES, GW])
        zb2 = ps("zb2", [LANES, GW])

        c16 = sem("c16")
        gsem = [sem("gsem0"), sem("gsem1")]
        watt = sem("watt")
        wpe = sem("wpe")
        wcp = sem("wcp")
        wsp = sem("wsp")
        wz = sem("wz")
        wlr = sem("wlr")
        wout = sem("wout")
        osem = [sem("osem0"), sem("osem1")]

        const_loads = [
            (xT, xT_d), (idxA, idxA_d), (idxB, idxB_d), (attw, att_d),
            (w1s, w1_d), (w2s, w2_d), (b1s, b1_d), (b2s, b2_d),
            (b1ns, b1n_d), (b2ns, b2n_d), (idents, ident_d),
        ]
        NCONST = len(const_loads)

        tableA = table[0:p.nA, :]
        tableB = table[p.nA:p.NPAD, :]

        # gather pieces per group (A + B split into PIECE_CH-chunk pieces),
        # and the cumulative per-parity gsem target after group g completes
        PIECE_CH = 8  # chunks per gather piece (1024 idxs; DGE ring limit)
        def npieces(n):
            return (int(n) + PIECE_CH - 1) // PIECE_CH
        n_gath = [npieces(p.nAg[g]) + npieces(p.nBg[g]) for g in range(NG)]
        gtarget = []
        par_tot = [0, 0]
        for g in range(NG):
            par_tot[g % 2] += n_gath[g]
            gtarget.append(16 * par_tot[g % 2])

        KREP = int(os.environ.get("KREP", "1"))
        NGG = KREP * NG
        NTT = KREP * NT
        # cumulative gather-piece gsem targets over global group index
        gtargetG = []
        par_totG = [0, 0]
        for gg in range(NGG):
            par_totG[gg % 2] += n_gath[gg % NG]
            gtargetG.append(16 * par_totG[gg % 2])

        with nc.Block() as block:

            @block.sync
            def _(sync):
                for dst_sb, src_d in const_loads:
                    sync.dma_start(dst_sb[:], src_d[:]).then_inc(c16, 16)
                if STAGE == -1:
                    sync.wait_ge(c16, 16 * NCONST)
                    return
                if STAGE == 0:
                    sync.wait_ge(gsem[0], 16 * n_gath[0])
                    sync.dma_start(
                        outT_d[:, 0:GW], gbuf[0][:, 0:GROUP, :],
                    ).then_inc(osem[0], 16)
                    sync.wait_ge(osem[0], 16)
                    return
                for gg in range(NGG):
                    g = gg % NG
                    if STAGE == 1:
                        sync.wait_ge(wcp, 4 * (gg + 1))
                        srcb = nhb[gg % 2]
                    elif STAGE == 2:
                        sync.wait_ge(wsp, gg + 1)
                        srcb = sumb[gg % 2]
                    elif STAGE == 3:
                        sync.wait_ge(wlr, gg + 1)
                        srcb = t1b[gg % 2]
                    else:
                        sync.wait_ge(wout, gg + 1)
                        srcb = outb[gg % 2]
                    sync.dma_start(
                        outT_d[:, g * GW:(g + 1) * GW],
                        srcb[:, :],
                    ).then_inc(osem[gg % 2], 16)
                sync.wait_ge(osem[0], 16 * ((NGG + 1) // 2))
                if NGG > 1:
                    sync.wait_ge(osem[1], 16 * (NGG // 2))

            @block.gpsimd
            def _(gpsimd):
                if STAGE == -1:
                    return
                gpsimd.load_library(mlp)
                gpsimd.wait_ge(c16, 16 * NCONST)
                for gg in range(NGG if STAGE > 0 else 1):
                    g = gg % NG
                    if gg >= 2:
                        gpsimd.wait_ge(wpe, 4 * (gg - 1))
                    nag = int(p.nAg[g])
                    nbg = int(p.nBg[g])
                    gb = gbuf[gg % 2]
                    for s0 in range(0, nag, PIECE_CH):
                        pc = min(PIECE_CH, nag - s0)
                        gpsimd.dma_gather(
                            gb[:, s0:s0 + pc, :], tableA,
                            idxA[:, (int(p.idxA_off[g]) + s0) * 8:
                                 (int(p.idxA_off[g]) + s0 + pc) * 8],
                            pc * LANES, pc * LANES, D,
                            queue_num=0, single_packet=False,
                        ).then_inc(gsem[gg % 2], 16)
                    for s0 in range(0, nbg, PIECE_CH):
                        pc = min(PIECE_CH, nbg - s0)
                        gpsimd.dma_gather(
                            gb[:, nag + s0:nag + s0 + pc, :], tableB,
                            idxB[:, (int(p.idxB_off[g]) + s0) * 8:
                                 (int(p.idxB_off[g]) + s0 + pc) * 8],
                            pc * LANES, pc * LANES, D,
                            queue_num=0, single_packet=False,
                        ).then_inc(gsem[gg % 2], 16)

            @block.vector
            def _(vector):
                if STAGE in (-1, 0):
                    return
                vector.wait_ge(c16, 16 * NCONST)

                def att_mul(gg):
                    g = gg % NG
                    nch = int(p.nAg[g] + p.nBg[g])
                    vector.wait_ge(gsem[gg % 2], gtargetG[gg])
                    gflat = gbuf[gg % 2][:, 0:nch, :]
                    a_ap = attw[:, int(p.att_off[g]):int(p.att_off[g]) + nch]
                    a_bc = bass.AP(
                        tensor=a_ap.tensor, offset=a_ap.offset,
                        ap=list(a_ap.ap) + [[0, D]],
                    )
                    nc.vector.tensor_tensor(
                        gflat, gflat, a_bc, mybir.AluOpType.mult
                    ).then_inc(watt, 1)

                def sum_prod(gg):
                    g = gg % NG
                    vector.wait_ge(wcp, 4 * (gg + 1))
                    if gg >= 2:
                        if STAGE >= 3:
                            vector.wait_ge(wz, gg - 1)
                        else:
                            vector.wait_ge(osem[gg % 2], 16 * (gg // 2))
                    xg = xT[:, g * GW:(g + 1) * GW]
                    nc.vector.tensor_tensor(
                        sumb[gg % 2][:, :], nhb[gg % 2][:, :], xg,
                        mybir.AluOpType.add)
                    nc.vector.tensor_tensor(
                        prodb[gg % 2][:, :], nhb[gg % 2][:, :], xg,
                        mybir.AluOpType.mult).then_inc(wsp, 1)

                def final_add(gg):
                    vector.wait_ge(wlr, gg + 1)
                    if gg >= 2:
                        vector.wait_ge(osem[gg % 2], 16 * (gg // 2))
                    nc.vector.tensor_tensor(
                        outb[gg % 2][:, :], t1b[gg % 2][:, :],
                        t2b[gg % 2][:, :], mybir.AluOpType.add)
                    nc.vector.tensor_tensor(
                        wb[:, :], u1b[gg % 2][:, :], u2b[gg % 2][:, :],
                        mybir.AluOpType.add)
                    nc.vector.drain()
                    nc.vector.tensor_tensor(
                        outb[gg % 2][:, :], outb[gg % 2][:, :], wb[:, :],
                        mybir.AluOpType.subtract)
                    nc.vector.drain().then_inc(wout, 1)

                att_mul(0)
                for gg in range(NGG):
                    if gg + 1 < NGG:
                        att_mul(gg + 1)
                    if STAGE >= 4 and gg >= 1:
                        final_add(gg - 1)
                    if STAGE >= 2:
                        sum_prod(gg)
                if STAGE >= 4:
                    final_add(NGG - 1)

            @block.tensor
            def _(tensor):
                if STAGE in (-1, 0):
                    return
                tensor.wait_ge(c16, 16 * NCONST)

                def tile_acc(tt):
                    tau = tt % NT
                    g = tau // GROUP
                    if tt % GROUP == 0:
                        tensor.wait_ge(watt, tt // GROUP + 1)
                    if tt >= 4:
                        tensor.wait_ge(wcp, tt - 3)
                    ka = int(p.K_A[tau])
                    kb = int(p.K_B[tau])
                    nag = int(p.nAg[g])
                    ab = int(p.a_base[tau])
                    bb = int(p.b_base[tau])
                    chunks = [ab + j for j in range(ka)] + \
                             [nag + bb + j for j in range(kb)]
                    gb = gbuf[(tt // GROUP) % 2]
                    pt = acc[tt % 4]
                    n = len(chunks)
                    for j, cidx in enumerate(chunks):
                        mm = nc.tensor.matmul(
                            pt[:, 0:D], gb[:, cidx, :], idents[:, :],
                            start=(j == 0), stop=(j == n - 1),
                            is_transpose=True,
                        )
                    mm.then_inc(wpe, 1)

                def zmm(gg):
                    tensor.wait_ge(wsp, gg + 1)
                    if gg >= 1:
                        tensor.wait_ge(wlr, gg)
                    nc.tensor.matmul(
                        zb1[:, :], w1s[:, :], sumb[gg % 2][:, :],
                        start=True, stop=True)
                    nc.tensor.matmul(
                        zb2[:, :], w2s[:, :], prodb[gg % 2][:, :],
                        start=True, stop=True).then_inc(wz, 1)

                for gg in range(NGG):
                    for tt in range(gg * GROUP, (gg + 1) * GROUP):
                        tile_acc(tt)
                    if STAGE >= 3 and gg >= 1:
                        zmm(gg - 1)
                if STAGE >= 3:
                    zmm(NGG - 1)

            @block.scalar
            def _(scalar):
                if STAGE in (-1, 0):
                    return
                scalar.wait_ge(c16, 16 * NCONST)

                def copies(gg):
                    for i in range(GROUP):
                        tt = gg * GROUP + i
                        scalar.wait_ge(wpe, tt + 1)
                        if gg >= 2 and i == 0:
                            if STAGE >= 2:
                                scalar.wait_ge(wsp, gg - 1)
                            else:
                                scalar.wait_ge(osem[gg % 2], 16 * (gg // 2))
                        nc.scalar.copy(
                            nhb[gg % 2][:, i * D:(i + 1) * D],
                            acc[tt % 4][:, 0:D],
                        ).then_inc(wcp, 1)

                def lrelu(gg):
                    scalar.wait_ge(wz, gg + 1)
                    if gg >= 2:
                        if STAGE >= 4:
                            scalar.wait_ge(wout, gg - 1)
                        else:
                            scalar.wait_ge(osem[gg % 2], 16 * (gg // 2))
                    nc.scalar.activation(
                        t1b[gg % 2][:, :], zb1[:, :], AF.Relu,
                        bias=b1s[:, 0:1], scale=1.0)
                    nc.scalar.activation(
                        u1b[gg % 2][:, :], zb1[:, :], AF.Relu,
                        bias=b1ns[:, 0:1], scale=-0.01)
                    nc.scalar.activation(
                        t2b[gg % 2][:, :], zb2[:, :], AF.Relu,
                        bias=b2s[:, 0:1], scale=1.0)
                    nc.scalar.activation(
                        u2b[gg % 2][:, :], zb2[:, :], AF.Relu,
                        bias=b2ns[:, 0:1], scale=-0.01,
                    ).then_inc(wlr, 1)

                for gg in range(NGG):
                    copies(gg)
                    if STAGE >= 3 and gg >= 1:
                        lrelu(gg - 1)
                if STAGE >= 3:
                    lrelu(NGG - 1)

    nc.compile()
    return nc


# --------------------------------------------------------------------------
# Entry point
# --------------------------------------------------------------------------

def _run(plan, W1, b1, W2, b2, n_cores, debug=False, trace=False,
         ident_bf16=False):
    nc = build_nc(plan, W1, b1, W2, b2, debug=debug, ident_bf16=ident_bf16)
    consts = {
        "W1t": np.ascontiguousarray(np.asarray(W1, np.float32).T),
        "W2t": np.ascontiguousarray(np.asarray(W2, np.float32).T),
        "b1c": np.asarray(b1, np.float32).reshape(D, 1).copy(),
        "b2c": np.asarray(b2, np.float32).reshape(D, 1).copy(),
        "b1n": (-0.01 * np.asarray(b1, np.float32)).reshape(D, 1).copy(),
        "b2n": (-0.01 * np.asarray(b2, np.float32)).reshape(D, 1).copy(),
        "ident": np.eye(D, dtype=(np.float32 if not ident_bf16 else None))
        if not ident_bf16 else None,
    }
    if ident_bf16:
        import ml_dtypes
        consts["ident"] = np.eye(D).astype(ml_dtypes.bfloat16)
    in_maps = []
    for c in range(n_cores):
        m = dict(plan.inputs[c])
        m.update(consts)
        in_maps.append(m)
    res = run_bass_kernel_spmd(nc, in_maps, core_ids=list(range(n_cores)),
                               trace=trace)
    return res


def assemble_output(plan, results):
    """results: list of per-core dicts with 'outT' -> full [N, D] output."""
    out = np.zeros((plan.NPAD, D), np.float32)
    for c in range(plan.n_cores):
        rows = np.asarray(results[c]["outT"]).T  # [NT*128, D]
        out[plan.node_ids[c]] = rows
    return out[:plan.n_nodes]


def kernel(entity_embed, att, W1, b1, W2, b2, src, dst):
    entity_embed = np.asarray(entity_embed, np.float32)
    att = np.asarray(att, np.float32)
    src = np.asarray(src).astype(np.int64)
    dst = np.asarray(dst).astype(np.int64)
    plan = make_plan(entity_embed, att, src, dst, n_cores=8)
    res = _run(plan, W1, b1, W2, b2, n_cores=8)
    return assemble_output(plan, res.results)


if __name__ == "__main__":
    # smoke test with random small data through the full-size path
    pass

